# revision 18
# baseline (speedup 1.0000x reference)
"""Trainium2 Bass kernel for nn_OmegaEntangle (E^T C E with entangle coefficients).

Math (validated vs reference, ~5e-3 rel err in bf16):
  p_i = sum_j v_ij^2 ; msum_i = sum_j v_ij ; m_i = msum_i / V
  C[i,j] = mask(i<j) * sqrt(p_i p_j) * (m_i + 1j*m_j) / sqrt(m_i^2 + m_j^2)
  out = E^T C E   (complex, E real)  ->  out_re = E^T Cr E, out_im = E^T Ci E

Key decomposition: with a_i = m_i*sqrt(p_i), b_i = sqrt(p_i) and
R_ij = mask(i<j)/sqrt(msum_i^2+msum_j^2)  (so  r~_ij = V * R_ij):
  Cr = diag(a) r~ diag(b)  ->  T_re = a ⊙ (R @ (E · bV)),  bV_j = V*sqrt(p_j)
  Ci = diag(b) r~ diag(a)  ->  T_im = b ⊙ (R @ (E · aV)),  aV_j = msum_j*sqrt(p_j)
  out_re = E^T T_re ; out_im = E^T T_im
Only ONE real-valued masked matrix R is built on device (4 [128,512] bf16
tiles, descending jt so each T-block completes one supply-step at a time);
only the 4 diagonal blocks of R need the triangular mask (upper off-diagonal
blocks are dense, lower blocks are skipped entirely -> 10 block-matmuls in
chain 1). The i-side diag scalings (a, b) fold into the PSUM->SBUF copies;
the j-side scalings (bV, aV) are pre-applied to the E column shard on the
host between the two launches (host already holds p/msum then).

Kernel A computes p/msum on the otherwise-idle tensor engine: the vuln shard
is sent transposed and ones-augmented; one long PSUM accumulation of
chunk^T @ chunk yields [65,64] whose diag is p and last row is msum
(host extracts). DMA-bound at ~15.5 us for the 4.26 MB bf16 shard.

All matmul operands and big DMA payloads are bf16 (host-cast; tolerance is
2e-2, this lands ~5.3e-3).

Sharding: data-parallel over the 2048 OUTPUT COLUMNS (256 per core), with the
p/m reduction row-sharded (64 rows per core). Two NEFF launches with a host
gather of the tiny [65,64] reduction results between them.
"""

import numpy as np
import ml_dtypes

import concourse.mybir as mybir
import concourse.tile as tile
from concourse import bacc
from concourse.bass_utils import run_bass_kernel_spmd
from concourse.dve_ops import RECIP_APPROX_FAST_CONSTS, RECIPROCAL_APPROX_FAST

D = 512          # number of domains
V = 32768        # vuln dim
S = 2048         # sup (embed) dim
NCORES = 8
ROWS_PER_CORE = D // NCORES          # 64
COLS_PER_CORE = S // NCORES          # 256
KT = D // 128                         # 4 contraction blocks
NS = S // 512                         # 4 chain-2 output column chunks
INV_V = 1.0 / V
BF = ml_dtypes.bfloat16

F32 = mybir.dt.float32
BF16 = mybir.dt.bfloat16
AF = mybir.ActivationFunctionType
ALU = mybir.AluOpType

# ---- tunables -------------------------------------------------------------
A_CHUNKS = [64, 64, 64, 48, 16]                # 128-col chunks per DMA tile


def _pad512(elems):
    """Round a bf16 element count up so the row pitch is a 512-byte multiple."""
    return ((elems * 2 + 511) // 512) * 512 // 2
A_BUFS = 4
A_WARMUP = 16
B_WARMUP = 16                                  # PE warm-up matmuls (HAM ramp)
B_WARM_FREE = 128

_CACHE = {}


def build_kernel_a():
    """Reduce kernel: gram-matrix trick on the tensor engine.

    Host sends the vuln shard TRANSPOSED and ones-augmented: 256 chunks of
    [128 j, 65] where cols 0:64 = v[j, row] and col 64 = 1. One long PSUM
    accumulation of chunk^T @ chunk[:, 0:64] yields [65, 64]: rows 0:64 are
    the gram matrix (diag = p), row 64 is msum. Host extracts diag/row.
    DMA-bound (~12 us); vector/scalar engines idle.
    """
    nc = bacc.Bacc("TRN2", target_bir_lowering=False, debug=False, num_devices=NCORES)

    NCH = V // 128                   # 256 chunks
    vins = [
        nc.dram_tensor(f"v{t}", [128, _pad512(ch * 65)], BF16, kind="ExternalInput")
        for t, ch in enumerate(A_CHUNKS)
    ]
    out_g = nc.dram_tensor("out_g", [65, 64], F32, kind="ExternalOutput")

    with tile.TileContext(nc) as tc:
        with (
            tc.tile_pool(name="vin", bufs=A_BUFS) as vin_pool,
            tc.tile_pool(name="small", bufs=1) as small_pool,
            tc.tile_pool(name="ps", bufs=1, space="PSUM") as ps_pool,
            tc.tile_pool(name="psw", bufs=1, space="PSUM") as psw_pool,
        ):
            # PE warm-up during preamble/first-tile DMA
            warm_b = small_pool.tile([128, 64], BF16, name="warm_b")
            nc.gpsimd.memset(warm_b[:], 0.001)
            ps_w = psw_pool.tile([64, 64], F32, name="ps_w")
            for i in range(A_WARMUP):
                nc.tensor.matmul(
                    ps_w[:], warm_b[:], warm_b[:],
                    start=(i == 0), stop=(i == A_WARMUP - 1),
                )

            vts = []
            maxw = _pad512(max(A_CHUNKS) * 65)
            for t, ch in enumerate(A_CHUNKS):
                w = _pad512(ch * 65)
                vt = vin_pool.tile([128, maxw], BF16, name=f"vt{t}", tag="vt")
                nc.sync.dma_start(vt[:, 0:w], vins[t][:])
                vts.append(vt)

            ps_g = ps_pool.tile([65, 64], F32, name="ps_g")
            done = 0
            for t, ch in enumerate(A_CHUNKS):
                for c in range(ch):
                    nc.tensor.matmul(
                        ps_g[:],
                        vts[t][:, 65 * c : 65 * c + 65],
                        vts[t][:, 65 * c : 65 * c + 64],
                        start=(done == 0),
                        stop=(done == NCH - 1),
                    )
                    done += 1

            gsb = small_pool.tile([65, 64], F32, name="gsb")
            nc.vector.tensor_copy(gsb[:], ps_g[:])
            nc.sync.dma_start(out_g[:], gsb[:])

    nc.compile()
    return nc


def build_kernel_b():
    """Main kernel: build R, two matmul chains, write transposed bf16 slabs."""
    nc = bacc.Bacc("TRN2", target_bir_lowering=False, debug=False, num_devices=NCORES)

    # pm_pp: per-partition layout, col kt = p[q+128kt], 4+kt = msum, 8+kt = msum^2
    pm_pp = nc.dram_tensor("pm_pp", [128, 3 * KT], F32, kind="ExternalInput")
    # msum^2 broadcast (bf16) and host-prescaled per-jt [E*bV | E*aV] blocks
    ms2_in = nc.dram_tensor("ms2", [128, D], BF16, kind="ExternalInput")
    ebea_in = nc.dram_tensor("ebea", [128, KT * 2 * COLS_PER_CORE], BF16,
                             kind="ExternalInput")
    efull = nc.dram_tensor("efull", [KT, 128, S], BF16, kind="ExternalInput")
    # transposed output slabs (host transposes back): out[:, cols] = slab.T
    out_re = nc.dram_tensor("out_re", [COLS_PER_CORE, S], BF16, kind="ExternalOutput")
    out_im = nc.dram_tensor("out_im", [COLS_PER_CORE, S], BF16, kind="ExternalOutput")

    rc = RECIP_APPROX_FAST_CONSTS
    CP = COLS_PER_CORE

    with tile.TileContext(nc) as tc:
        with (
            tc.tile_pool(name="epool", bufs=1) as e_pool,
            tc.tile_pool(name="small", bufs=1) as small_pool,
            tc.tile_pool(name="hb", bufs=2) as h_pool,
            tc.tile_pool(name="ost", bufs=4) as o_pool,
            tc.tile_pool(name="psA", bufs=1, space="PSUM") as psA,
            tc.tile_pool(name="psB", bufs=4, space="PSUM") as psB,
        ):
            # -------- input DMAs --------------------------------------------
            # sync: small early tensors; gpsimd: e0/e1; scalar (after h's): e2/e3
            pp = small_pool.tile([128, 3 * KT], F32, name="pp")
            nc.sync.dma_start(pp[:], pm_pp[:])
            ms2_t = small_pool.tile([128, D], BF16, name="ms2_t")
            nc.sync.dma_start(ms2_t[:], ms2_in[:])
            ebea_t = small_pool.tile([128, KT * 2 * CP], BF16, name="ebea_t")
            nc.sync.dma_start(ebea_t[:], ebea_in[:])
            ms2_bc = ms2_t[:, 0:D]
            ebea = [
                ebea_t[:, 2 * CP * jt : 2 * CP * (jt + 1)] for jt in range(KT)
            ]

            e_sb = [
                e_pool.tile([128, S], BF16, name=f"e{kt}", tag=f"e{kt}")
                for kt in range(KT)
            ]

            # -------- PE warm-up (HAM ramp) during DMA/derivation -----------
            warm_b = small_pool.tile([128, B_WARM_FREE], BF16, name="warm_b")
            nc.gpsimd.memset(warm_b[:], 0.001)
            for kt in [3, 2, 1, 0]:
                nc.sync.dma_start(e_sb[kt][:], efull[kt])
            ps_w = psB.tile([128, 512], F32, name="ps_w", tag="o")
            for i in range(B_WARMUP):
                nc.tensor.matmul(
                    ps_w[:, 0:B_WARM_FREE], warm_b[:], warm_b[:],
                    start=(i == 0), stop=(i == B_WARMUP - 1),
                )

            # -------- tiny derived vectors ----------------------------------
            # b4n = sqrt(p); a4c = msum*sqrt(p)/V  (t_sb diag scalings)
            b4n = small_pool.tile([128, KT], F32, name="b4n")
            nc.scalar.activation(b4n[:], pp[:, 0:KT], AF.Sqrt)
            a4c = small_pool.tile([128, KT], F32, name="a4c")
            nc.vector.scalar_tensor_tensor(
                a4c[:], pp[:, KT : 2 * KT], INV_V, b4n[:], op0=ALU.mult, op1=ALU.mult
            )

            # -------- R build (descending jt) -------------------------------
            # per jt: scalar h = sqrt(ms2_i + ms2_j), vector recip -> bf16,
            # gpsimd masks the diagonal block.
            rt, rd = [None] * KT, [None] * KT
            for jt in [3, 2, 1, 0]:
                h = h_pool.tile([128, D], F32, name="h", tag="h")
                nc.scalar.activation(
                    h[:], ms2_bc, AF.Sqrt,
                    bias=pp[:, 2 * KT + jt : 2 * KT + jt + 1], scale=1.0,
                )
                rtj = e_pool.tile([128, D], BF16, name=f"rt{jt}", tag=f"rt{jt}")
                nc.vector._custom_dve(
                    RECIPROCAL_APPROX_FAST, out=rtj[:], in0=h[:],
                    s0=rc["s0"], s1=rc["s1"], imm2=rc["imm2"],
                )
                rt[jt] = rtj
                rdj = e_pool.tile([128, 128], BF16, name=f"rd{jt}", tag=f"rd{jt}")
                nc.gpsimd.affine_select(
                    out=rdj[:], in_=rtj[:, 128 * jt : 128 * (jt + 1)],
                    pattern=[[-1, 128]], compare_op=ALU.is_gt,
                    fill=0.0, base=0, channel_multiplier=1,
                )
                rd[jt] = rdj

            # -------- chain 1: T-blocks = R @ [E·bV | E·aV] -----------------
            ps_ts = [
                psA.tile([128, 2 * CP], F32, name=f"ps_t{it}", tag=f"t{it}", bufs=1)
                for it in range(KT)
            ]
            # descending jt: every group starts at jt==3; the diag (it==jt)
            # is each group's LAST contribution (stop), emitted after the
            # off-diagonal blocks so it can wait on rd[jt] without stalling.
            t_sb = [None] * KT
            for jt in [3, 2, 1, 0]:
                for it in range(jt):
                    nc.tensor.matmul(
                        ps_ts[it][:], rt[jt][:, 128 * it : 128 * (it + 1)], ebea[jt],
                        start=(jt == 3), stop=False,
                    )
                nc.tensor.matmul(
                    ps_ts[jt][:], rd[jt][:], ebea[jt],
                    start=(jt == 3), stop=True,
                )
                # T-block jt is complete: copy to SBUF with diag scalings
                tsb = e_pool.tile([128, 2 * CP], BF16, name=f"tsb{jt}", tag=f"tsb{jt}")
                nc.scalar.activation(
                    tsb[:, 0:CP], ps_ts[jt][:, 0:CP], AF.Copy,
                    scale=a4c[:, jt : jt + 1],
                )
                nc.vector.tensor_scalar_mul(
                    tsb[:, CP : 2 * CP], ps_ts[jt][:, CP : 2 * CP],
                    b4n[:, jt : jt + 1],
                )
                t_sb[jt] = tsb

            # -------- chain 2: out^T slabs = T^T @ E ------------------------
            # groups: (re, mc0), (re, mc1), (im, mc0), (im, mc1)
            groups = [
                (0, 0, out_re), (0, 1, out_re), (1, 0, out_im), (1, 1, out_im),
            ]
            cnt = 0
            for gi, (part, mc, out_t) in enumerate(groups):
                c0 = part * CP + mc * 128
                if gi % 2 == 0:
                    pso = [
                        psB.tile([128, 512], F32, name=f"pso{part}{mc}{sn}", tag="o")
                        for sn in range(NS)
                    ]
                else:
                    pso = [
                        psA.tile([128, 2 * CP], F32, name=f"pso{part}{mc}{sn}",
                                 tag=f"t{sn}", bufs=1)
                        for sn in range(NS)
                    ]
                for idx_it, it in enumerate([3, 2, 1, 0]):
                    for sn in range(NS):
                        nc.tensor.matmul(
                            pso[sn][:],
                            t_sb[it][:, c0 : c0 + 128],
                            e_sb[it][:, 512 * sn : 512 * (sn + 1)],
                            start=(idx_it == 0), stop=(idx_it == KT - 1),
                        )
                for sn in range(NS):
                    osb = o_pool.tile([128, 512], BF16, name="osb", tag="osb")
                    if cnt % 2 == 0:
                        nc.scalar.copy(osb[:], pso[sn][:])
                    else:
                        nc.vector.tensor_copy(osb[:], pso[sn][:])
                    nc.sync.dma_start(
                        out_t[mc * 128 : (mc + 1) * 128, 512 * sn : 512 * (sn + 1)],
                        osb[:],
                    )
                    cnt += 1

    nc.compile()
    return nc


def _prepare_a_in_maps(vulns):
    vulns = np.asarray(vulns)
    NCH = V // 128
    in_maps = []
    for c in range(NCORES):
        vsh = vulns[c * ROWS_PER_CORE : (c + 1) * ROWS_PER_CORE]
        aug = np.empty((V, 65), dtype=BF)
        aug[:, 0:64] = vsh.T.astype(BF)
        aug[:, 64] = np.asarray(1.0, dtype=BF)
        # [NCH, 128, 65] -> [128, NCH*65] with chunk k at free cols [65k, 65k+65)
        v128 = aug.reshape(NCH, 128, 65).transpose(1, 0, 2).reshape(128, NCH * 65)
        m = {}
        off = 0
        for t, ch in enumerate(A_CHUNKS):
            w = ch * 65
            wp = _pad512(w)
            arr = np.zeros((128, wp), dtype=BF)
            arr[:, 0:w] = v128[:, off : off + w]
            m[f"v{t}"] = arr
            off += w
        in_maps.append(m)
    return in_maps


def _prepare_b_in_maps(embed_table, domain_ids, p_full, msum_full):
    embed_table = np.asarray(embed_table, dtype=np.float32)
    domain_ids = np.asarray(domain_ids).astype(np.int64)
    E = np.ascontiguousarray(embed_table[domain_ids])          # [512, 2048] f32
    e4 = np.ascontiguousarray(E.astype(BF).reshape(KT, 128, S))
    p64 = p_full.astype(np.float64)
    ms64 = msum_full.astype(np.float64)
    ms2 = (ms64 ** 2).astype(np.float32)
    bV = (float(V) * np.sqrt(p64)).astype(np.float32)          # V*sqrt(p)
    aV = (ms64 * np.sqrt(p64)).astype(np.float32)              # msum*sqrt(p)
    # per-partition layout [128, 12]
    pm_pp = np.empty((128, 3 * KT), dtype=np.float32)
    pm_pp[:, 0:KT] = p_full.reshape(KT, 128).T
    pm_pp[:, KT : 2 * KT] = msum_full.reshape(KT, 128).T
    pm_pp[:, 2 * KT : 3 * KT] = ms2.reshape(KT, 128).T
    CP = COLS_PER_CORE
    ms2_b = np.ascontiguousarray(np.broadcast_to(ms2.astype(BF), (128, D)))
    in_maps = []
    for c in range(NCORES):
        cols = slice(c * CP, (c + 1) * CP)
        Ec = E[:, cols]                                        # [512, 256] f32
        Eb = (Ec * bV[:, None]).astype(BF).reshape(KT, 128, CP)
        Ea = (Ec * aV[:, None]).astype(BF).reshape(KT, 128, CP)
        ebea = np.empty((128, KT * 2 * CP), dtype=BF)
        for jt in range(KT):
            ebea[:, 2 * CP * jt : 2 * CP * jt + CP] = Eb[jt]
            ebea[:, 2 * CP * jt + CP : 2 * CP * (jt + 1)] = Ea[jt]
        in_maps.append(
            {"pm_pp": pm_pp, "ms2": ms2_b, "ebea": np.ascontiguousarray(ebea),
             "efull": e4}
        )
    return in_maps


def kernel(vulns, embed_table, domain_ids, _trace=False):
    if "nc_a" not in _CACHE:
        _CACHE["nc_a"] = build_kernel_a()
    if "nc_b" not in _CACHE:
        _CACHE["nc_b"] = build_kernel_b()

    res_a = run_bass_kernel_spmd(
        _CACHE["nc_a"], _prepare_a_in_maps(vulns),
        core_ids=list(range(NCORES)), trace=_trace,
    )
    _CACHE["res_a"] = res_a
    idx = np.arange(ROWS_PER_CORE)
    p_full = np.concatenate(
        [res_a.results[c]["out_g"][idx, idx] for c in range(NCORES)]
    )
    msum_full = np.concatenate(
        [res_a.results[c]["out_g"][64, :] for c in range(NCORES)]
    )

    res_b = run_bass_kernel_spmd(
        _CACHE["nc_b"], _prepare_b_in_maps(embed_table, domain_ids, p_full, msum_full),
        core_ids=list(range(NCORES)), trace=_trace,
    )
    _CACHE["res_b"] = res_b

    out = np.empty((S, S), dtype=np.complex64)
    for c in range(NCORES):
        r = res_b.results[c]
        sl = slice(c * COLS_PER_CORE, (c + 1) * COLS_PER_CORE)
        out[:, sl] = (
            r["out_re"].astype(np.float32).T
            + 1j * r["out_im"].astype(np.float32).T
        )
    return out


if __name__ == "__main__":
    rng = np.random.default_rng(0)
    v = rng.standard_normal((D, V), dtype=np.float32)
    et = rng.standard_normal((D, S), dtype=np.float32)
    ids = np.arange(D, dtype=np.int32)
    out = kernel(v, et, ids)
    print(out.shape, out.dtype)


# revision 19
# speedup vs baseline: 1.0348x; 1.0348x over previous
"""Trainium2 Bass kernel for nn_OmegaEntangle (E^T C E with entangle coefficients).

Math (validated vs reference, ~5e-3 rel err in bf16):
  p_i = sum_j v_ij^2 ; msum_i = sum_j v_ij ; m_i = msum_i / V
  C[i,j] = mask(i<j) * sqrt(p_i p_j) * (m_i + 1j*m_j) / sqrt(m_i^2 + m_j^2)
  out = E^T C E   (complex, E real)  ->  out_re = E^T Cr E, out_im = E^T Ci E

Key decomposition: with a_i = m_i*sqrt(p_i), b_i = sqrt(p_i) and
R_ij = mask(i<j)/sqrt(msum_i^2+msum_j^2)  (so  r~_ij = V * R_ij):
  Cr = diag(a) r~ diag(b)  ->  T_re = a ⊙ (R @ (E · bV)),  bV_j = V*sqrt(p_j)
  Ci = diag(b) r~ diag(a)  ->  T_im = b ⊙ (R @ (E · aV)),  aV_j = msum_j*sqrt(p_j)
  out_re = E^T T_re ; out_im = E^T T_im
Only ONE real-valued masked matrix R is built on device (4 [128,512] bf16
tiles, descending jt so each T-block completes one supply-step at a time);
only the 4 diagonal blocks of R need the triangular mask (upper off-diagonal
blocks are dense, lower blocks are skipped entirely -> 10 block-matmuls in
chain 1). The i-side diag scalings (a, b) fold into the PSUM->SBUF copies;
the j-side scalings (bV, aV) are pre-applied to the E column shard on the
host between the two launches (host already holds p/msum then).

Kernel A computes p/msum on the otherwise-idle tensor engine: the vuln shard
is sent transposed and ones-augmented; one long PSUM accumulation of
chunk^T @ chunk yields [65,64] whose diag is p and last row is msum
(host extracts). DMA-bound at ~15.5 us for the 4.26 MB bf16 shard.

All matmul operands and big DMA payloads are bf16 (host-cast; tolerance is
2e-2, this lands ~5.3e-3).

Sharding: data-parallel over the 2048 OUTPUT COLUMNS (256 per core), with the
p/m reduction row-sharded (64 rows per core). Two NEFF launches with a host
gather of the tiny [65,64] reduction results between them.
"""

import numpy as np
import ml_dtypes

import concourse.mybir as mybir
import concourse.tile as tile
from concourse import bacc
from concourse.bass_utils import run_bass_kernel_spmd
from concourse.dve_ops import RECIP_APPROX_FAST_CONSTS, RECIPROCAL_APPROX_FAST

D = 512          # number of domains
V = 32768        # vuln dim
S = 2048         # sup (embed) dim
NCORES = 8
ROWS_PER_CORE = D // NCORES          # 64
COLS_PER_CORE = S // NCORES          # 256
KT = D // 128                         # 4 contraction blocks
NS = S // 512                         # 4 chain-2 output column chunks
INV_V = 1.0 / V
BF = ml_dtypes.bfloat16

F32 = mybir.dt.float32
BF16 = mybir.dt.bfloat16
AF = mybir.ActivationFunctionType
ALU = mybir.AluOpType

# ---- tunables -------------------------------------------------------------
A_CHUNKS = [64, 64, 64, 48, 16]                # 128-col chunks per DMA tile


def _pad512(elems):
    """Round a bf16 element count up so the row pitch is a 512-byte multiple."""
    return ((elems * 2 + 511) // 512) * 512 // 2
A_BUFS = 4
A_WARMUP = 16
B_WARMUP = 16                                  # PE warm-up matmuls (HAM ramp)
B_WARM_FREE = 128

_CACHE = {}


def build_kernel_a():
    """Reduce kernel: gram-matrix trick on the tensor engine.

    Host sends the vuln shard TRANSPOSED and ones-augmented: 256 chunks of
    [128 j, 65] where cols 0:64 = v[j, row] and col 64 = 1. One long PSUM
    accumulation of chunk^T @ chunk[:, 0:64] yields [65, 64]: rows 0:64 are
    the gram matrix (diag = p), row 64 is msum. Host extracts diag/row.
    DMA-bound (~12 us); vector/scalar engines idle.
    """
    nc = bacc.Bacc("TRN2", target_bir_lowering=False, debug=False, num_devices=NCORES)

    NCH = V // 128                   # 256 chunks
    vins = [
        nc.dram_tensor(f"v{t}", [128, _pad512(ch * 65)], BF16, kind="ExternalInput")
        for t, ch in enumerate(A_CHUNKS)
    ]
    out_g = nc.dram_tensor("out_g", [65, 64], F32, kind="ExternalOutput")

    with tile.TileContext(nc) as tc:
        with (
            tc.tile_pool(name="vin", bufs=A_BUFS) as vin_pool,
            tc.tile_pool(name="small", bufs=1) as small_pool,
            tc.tile_pool(name="ps", bufs=1, space="PSUM") as ps_pool,
            tc.tile_pool(name="psw", bufs=1, space="PSUM") as psw_pool,
        ):
            # PE warm-up during preamble/first-tile DMA
            warm_b = small_pool.tile([128, 64], BF16, name="warm_b")
            nc.gpsimd.memset(warm_b[:], 0.001)
            ps_w = psw_pool.tile([64, 64], F32, name="ps_w")
            for i in range(A_WARMUP):
                nc.tensor.matmul(
                    ps_w[:], warm_b[:], warm_b[:],
                    start=(i == 0), stop=(i == A_WARMUP - 1),
                )

            vts = []
            maxw = _pad512(max(A_CHUNKS) * 65)
            for t, ch in enumerate(A_CHUNKS):
                w = _pad512(ch * 65)
                vt = vin_pool.tile([128, maxw], BF16, name=f"vt{t}", tag="vt")
                nc.sync.dma_start(vt[:, 0:w], vins[t][:])
                vts.append(vt)

            ps_g = ps_pool.tile([65, 64], F32, name="ps_g")
            done = 0
            for t, ch in enumerate(A_CHUNKS):
                for c in range(ch):
                    nc.tensor.matmul(
                        ps_g[:],
                        vts[t][:, 65 * c : 65 * c + 65],
                        vts[t][:, 65 * c : 65 * c + 64],
                        start=(done == 0),
                        stop=(done == NCH - 1),
                    )
                    done += 1

            gsb = small_pool.tile([65, 64], F32, name="gsb")
            nc.vector.tensor_copy(gsb[:], ps_g[:])
            nc.sync.dma_start(out_g[:], gsb[:])

    nc.compile()
    return nc


def build_kernel_b():
    """Main kernel: build R, two matmul chains, write transposed bf16 slabs."""
    nc = bacc.Bacc("TRN2", target_bir_lowering=False, debug=False, num_devices=NCORES)

    # pm_pp: per-partition layout, col kt = p[q+128kt], 4+kt = msum, 8+kt = msum^2
    pm_pp = nc.dram_tensor("pm_pp", [128, 3 * KT], F32, kind="ExternalInput")
    # msum^2 broadcast (bf16) and host-prescaled per-jt [E*bV | E*aV] blocks
    ms2_in = nc.dram_tensor("ms2", [128, D], BF16, kind="ExternalInput")
    ebea_in = nc.dram_tensor("ebea", [128, KT * 2 * COLS_PER_CORE], BF16,
                             kind="ExternalInput")
    efull = nc.dram_tensor("efull", [KT, 128, S], BF16, kind="ExternalInput")
    # transposed output slabs (host transposes back): out[:, cols] = slab.T
    out_re = nc.dram_tensor("out_re", [COLS_PER_CORE, S], BF16, kind="ExternalOutput")
    out_im = nc.dram_tensor("out_im", [COLS_PER_CORE, S], BF16, kind="ExternalOutput")

    rc = RECIP_APPROX_FAST_CONSTS
    CP = COLS_PER_CORE

    with tile.TileContext(nc) as tc:
        with (
            tc.tile_pool(name="epool", bufs=1) as e_pool,
            tc.tile_pool(name="small", bufs=1) as small_pool,
            tc.tile_pool(name="hb", bufs=2) as h_pool,
            tc.tile_pool(name="ost", bufs=4) as o_pool,
            tc.tile_pool(name="psA", bufs=1, space="PSUM") as psA,
            tc.tile_pool(name="psB", bufs=4, space="PSUM") as psB,
        ):
            # -------- input DMAs --------------------------------------------
            # sync: small early tensors; gpsimd: e0/e1; scalar (after h's): e2/e3
            pp = small_pool.tile([128, 3 * KT], F32, name="pp")
            nc.sync.dma_start(pp[:], pm_pp[:])
            ms2_t = small_pool.tile([128, D], BF16, name="ms2_t")
            nc.sync.dma_start(ms2_t[:], ms2_in[:])
            ebea_t = small_pool.tile([128, KT * 2 * CP], BF16, name="ebea_t")
            nc.sync.dma_start(ebea_t[:], ebea_in[:])
            ms2_bc = ms2_t[:, 0:D]
            ebea = [
                ebea_t[:, 2 * CP * jt : 2 * CP * (jt + 1)] for jt in range(KT)
            ]

            e_sb = [
                e_pool.tile([128, S], BF16, name=f"e{kt}", tag=f"e{kt}")
                for kt in range(KT)
            ]

            # -------- PE warm-up (HAM ramp) during DMA/derivation -----------
            warm_b = small_pool.tile([128, B_WARM_FREE], BF16, name="warm_b")
            nc.gpsimd.memset(warm_b[:], 0.001)
            for kt in [3, 2, 1, 0]:
                nc.sync.dma_start(e_sb[kt][:], efull[kt])
            ps_w = psB.tile([128, 512], F32, name="ps_w", tag="o")
            for i in range(B_WARMUP):
                nc.tensor.matmul(
                    ps_w[:, 0:B_WARM_FREE], warm_b[:], warm_b[:],
                    start=(i == 0), stop=(i == B_WARMUP - 1),
                )

            # -------- tiny derived vectors ----------------------------------
            # b4n = sqrt(p); a4c = msum*sqrt(p)/V  (t_sb diag scalings)
            b4n = small_pool.tile([128, KT], F32, name="b4n")
            nc.scalar.activation(b4n[:], pp[:, 0:KT], AF.Sqrt)
            a4c = small_pool.tile([128, KT], F32, name="a4c")
            nc.vector.scalar_tensor_tensor(
                a4c[:], pp[:, KT : 2 * KT], INV_V, b4n[:], op0=ALU.mult, op1=ALU.mult
            )

            # -------- R build (descending jt) -------------------------------
            # per jt: scalar h = sqrt(ms2_i + ms2_j), vector recip -> bf16,
            # gpsimd masks the diagonal block.
            rt, rd = [None] * KT, [None] * KT
            for jt in [3, 2, 1, 0]:
                h = h_pool.tile([128, D], F32, name="h", tag="h")
                nc.scalar.activation(
                    h[:], ms2_bc, AF.Sqrt,
                    bias=pp[:, 2 * KT + jt : 2 * KT + jt + 1], scale=1.0,
                )
                rtj = e_pool.tile([128, D], BF16, name=f"rt{jt}", tag=f"rt{jt}")
                nc.vector._custom_dve(
                    RECIPROCAL_APPROX_FAST, out=rtj[:], in0=h[:],
                    s0=rc["s0"], s1=rc["s1"], imm2=rc["imm2"],
                )
                rt[jt] = rtj
                rdj = e_pool.tile([128, 128], BF16, name=f"rd{jt}", tag=f"rd{jt}")
                nc.gpsimd.affine_select(
                    out=rdj[:], in_=rtj[:, 128 * jt : 128 * (jt + 1)],
                    pattern=[[-1, 128]], compare_op=ALU.is_gt,
                    fill=0.0, base=0, channel_multiplier=1,
                )
                rd[jt] = rdj

            # -------- chain 1: T-blocks = R @ [E·bV | E·aV] -----------------
            ps_ts = [
                psA.tile([128, 2 * CP], F32, name=f"ps_t{it}", tag=f"t{it}", bufs=1)
                for it in range(KT)
            ]
            # descending jt: every group starts at jt==3; the diag (it==jt)
            # is each group's LAST contribution (stop), emitted after the
            # off-diagonal blocks so it can wait on rd[jt] without stalling.
            t_sb = [None] * KT
            for jt in [3, 2, 1, 0]:
                for it in range(jt):
                    nc.tensor.matmul(
                        ps_ts[it][:], rt[jt][:, 128 * it : 128 * (it + 1)], ebea[jt],
                        start=(jt == 3), stop=False,
                    )
                nc.tensor.matmul(
                    ps_ts[jt][:], rd[jt][:], ebea[jt],
                    start=(jt == 3), stop=True,
                )
                # T-block jt is complete: copy to SBUF with diag scalings
                tsb = e_pool.tile([128, 2 * CP], BF16, name=f"tsb{jt}", tag=f"tsb{jt}")
                nc.scalar.activation(
                    tsb[:, 0:CP], ps_ts[jt][:, 0:CP], AF.Copy,
                    scale=a4c[:, jt : jt + 1],
                )
                nc.vector.tensor_scalar_mul(
                    tsb[:, CP : 2 * CP], ps_ts[jt][:, CP : 2 * CP],
                    b4n[:, jt : jt + 1],
                )
                t_sb[jt] = tsb

            # -------- chain 2: out^T slabs = T^T @ E ------------------------
            # groups: (re, mc0), (re, mc1), (im, mc0), (im, mc1)
            groups = [
                (0, 0, out_re), (0, 1, out_re), (1, 0, out_im), (1, 1, out_im),
            ]
            cnt = 0
            for gi, (part, mc, out_t) in enumerate(groups):
                c0 = part * CP + mc * 128
                if gi % 2 == 0:
                    pso = [
                        psB.tile([128, 512], F32, name=f"pso{part}{mc}{sn}", tag="o")
                        for sn in range(NS)
                    ]
                else:
                    pso = [
                        psA.tile([128, 2 * CP], F32, name=f"pso{part}{mc}{sn}",
                                 tag=f"t{sn}", bufs=1)
                        for sn in range(NS)
                    ]
                for idx_it, it in enumerate([3, 2, 1, 0]):
                    for sn in range(NS):
                        nc.tensor.matmul(
                            pso[sn][:],
                            t_sb[it][:, c0 : c0 + 128],
                            e_sb[it][:, 512 * sn : 512 * (sn + 1)],
                            start=(idx_it == 0), stop=(idx_it == KT - 1),
                        )
                osb = o_pool.tile([128, S], BF16, name="osb", tag="osb")
                for sn in range(NS):
                    if cnt % 2 == 0:
                        nc.scalar.copy(osb[:, 512 * sn : 512 * (sn + 1)], pso[sn][:])
                    else:
                        nc.vector.tensor_copy(
                            osb[:, 512 * sn : 512 * (sn + 1)], pso[sn][:]
                        )
                    cnt += 1
                nc.sync.dma_start(out_t[mc * 128 : (mc + 1) * 128, :], osb[:])

    nc.compile()
    return nc


def _prepare_a_in_maps(vulns):
    vulns = np.asarray(vulns)
    NCH = V // 128
    in_maps = []
    for c in range(NCORES):
        vsh = vulns[c * ROWS_PER_CORE : (c + 1) * ROWS_PER_CORE]
        aug = np.empty((V, 65), dtype=BF)
        aug[:, 0:64] = vsh.T.astype(BF)
        aug[:, 64] = np.asarray(1.0, dtype=BF)
        # [NCH, 128, 65] -> [128, NCH*65] with chunk k at free cols [65k, 65k+65)
        v128 = aug.reshape(NCH, 128, 65).transpose(1, 0, 2).reshape(128, NCH * 65)
        m = {}
        off = 0
        for t, ch in enumerate(A_CHUNKS):
            w = ch * 65
            wp = _pad512(w)
            arr = np.zeros((128, wp), dtype=BF)
            arr[:, 0:w] = v128[:, off : off + w]
            m[f"v{t}"] = arr
            off += w
        in_maps.append(m)
    return in_maps


def _prepare_b_in_maps(embed_table, domain_ids, p_full, msum_full):
    embed_table = np.asarray(embed_table, dtype=np.float32)
    domain_ids = np.asarray(domain_ids).astype(np.int64)
    E = np.ascontiguousarray(embed_table[domain_ids])          # [512, 2048] f32
    e4 = np.ascontiguousarray(E.astype(BF).reshape(KT, 128, S))
    p64 = p_full.astype(np.float64)
    ms64 = msum_full.astype(np.float64)
    ms2 = (ms64 ** 2).astype(np.float32)
    bV = (float(V) * np.sqrt(p64)).astype(np.float32)          # V*sqrt(p)
    aV = (ms64 * np.sqrt(p64)).astype(np.float32)              # msum*sqrt(p)
    # per-partition layout [128, 12]
    pm_pp = np.empty((128, 3 * KT), dtype=np.float32)
    pm_pp[:, 0:KT] = p_full.reshape(KT, 128).T
    pm_pp[:, KT : 2 * KT] = msum_full.reshape(KT, 128).T
    pm_pp[:, 2 * KT : 3 * KT] = ms2.reshape(KT, 128).T
    CP = COLS_PER_CORE
    ms2_b = np.ascontiguousarray(np.broadcast_to(ms2.astype(BF), (128, D)))
    in_maps = []
    for c in range(NCORES):
        cols = slice(c * CP, (c + 1) * CP)
        Ec = E[:, cols]                                        # [512, 256] f32
        Eb = (Ec * bV[:, None]).astype(BF).reshape(KT, 128, CP)
        Ea = (Ec * aV[:, None]).astype(BF).reshape(KT, 128, CP)
        ebea = np.empty((128, KT * 2 * CP), dtype=BF)
        for jt in range(KT):
            ebea[:, 2 * CP * jt : 2 * CP * jt + CP] = Eb[jt]
            ebea[:, 2 * CP * jt + CP : 2 * CP * (jt + 1)] = Ea[jt]
        in_maps.append(
            {"pm_pp": pm_pp, "ms2": ms2_b, "ebea": np.ascontiguousarray(ebea),
             "efull": e4}
        )
    return in_maps


def kernel(vulns, embed_table, domain_ids, _trace=False):
    if "nc_a" not in _CACHE:
        _CACHE["nc_a"] = build_kernel_a()
    if "nc_b" not in _CACHE:
        _CACHE["nc_b"] = build_kernel_b()

    res_a = run_bass_kernel_spmd(
        _CACHE["nc_a"], _prepare_a_in_maps(vulns),
        core_ids=list(range(NCORES)), trace=_trace,
    )
    _CACHE["res_a"] = res_a
    idx = np.arange(ROWS_PER_CORE)
    p_full = np.concatenate(
        [res_a.results[c]["out_g"][idx, idx] for c in range(NCORES)]
    )
    msum_full = np.concatenate(
        [res_a.results[c]["out_g"][64, :] for c in range(NCORES)]
    )

    res_b = run_bass_kernel_spmd(
        _CACHE["nc_b"], _prepare_b_in_maps(embed_table, domain_ids, p_full, msum_full),
        core_ids=list(range(NCORES)), trace=_trace,
    )
    _CACHE["res_b"] = res_b

    out = np.empty((S, S), dtype=np.complex64)
    for c in range(NCORES):
        r = res_b.results[c]
        sl = slice(c * COLS_PER_CORE, (c + 1) * COLS_PER_CORE)
        out[:, sl] = (
            r["out_re"].astype(np.float32).T
            + 1j * r["out_im"].astype(np.float32).T
        )
    return out


if __name__ == "__main__":
    rng = np.random.default_rng(0)
    v = rng.standard_normal((D, V), dtype=np.float32)
    et = rng.standard_normal((D, S), dtype=np.float32)
    ids = np.arange(D, dtype=np.int32)
    out = kernel(v, et, ids)
    print(out.shape, out.dtype)


# revision 20
# speedup vs baseline: 1.0467x; 1.0115x over previous
"""Trainium2 Bass kernel for nn_OmegaEntangle (E^T C E with entangle coefficients).

Math (validated vs reference, ~5e-3 rel err in bf16):
  p_i = sum_j v_ij^2 ; msum_i = sum_j v_ij ; m_i = msum_i / V
  C[i,j] = mask(i<j) * sqrt(p_i p_j) * (m_i + 1j*m_j) / sqrt(m_i^2 + m_j^2)
  out = E^T C E   (complex, E real)  ->  out_re = E^T Cr E, out_im = E^T Ci E

Key decomposition: with a_i = m_i*sqrt(p_i), b_i = sqrt(p_i) and
R_ij = mask(i<j)/sqrt(msum_i^2+msum_j^2)  (so  r~_ij = V * R_ij):
  Cr = diag(a) r~ diag(b)  ->  T_re = a ⊙ (R @ (E · bV)),  bV_j = V*sqrt(p_j)
  Ci = diag(b) r~ diag(a)  ->  T_im = b ⊙ (R @ (E · aV)),  aV_j = msum_j*sqrt(p_j)
  out_re = E^T T_re ; out_im = E^T T_im
Only ONE real-valued masked matrix R is built on device (4 [128,512] bf16
tiles, descending jt so each T-block completes one supply-step at a time);
only the 4 diagonal blocks of R need the triangular mask (upper off-diagonal
blocks are dense, lower blocks are skipped entirely -> 10 block-matmuls in
chain 1). The i-side diag scalings (a, b) fold into the PSUM->SBUF copies;
the j-side scalings (bV, aV) are pre-applied to the E column shard on the
host between the two launches (host already holds p/msum then).

Kernel A computes p/msum on the otherwise-idle tensor engine: the vuln shard
is sent transposed and ones-augmented; one long PSUM accumulation of
chunk^T @ chunk yields [65,64] whose diag is p and last row is msum
(host extracts). DMA-bound at ~15.5 us for the 4.26 MB bf16 shard.

All matmul operands and big DMA payloads are bf16 (host-cast; tolerance is
2e-2, this lands ~5.3e-3).

Sharding: data-parallel over the 2048 OUTPUT COLUMNS (256 per core), with the
p/m reduction row-sharded (64 rows per core). Two NEFF launches with a host
gather of the tiny [65,64] reduction results between them.
"""

import numpy as np
import ml_dtypes

import concourse.mybir as mybir
import concourse.tile as tile
from concourse import bacc
from concourse.bass_utils import run_bass_kernel_spmd
from concourse.dve_ops import RECIP_APPROX_FAST_CONSTS, RECIPROCAL_APPROX_FAST

D = 512          # number of domains
V = 32768        # vuln dim
S = 2048         # sup (embed) dim
NCORES = 8
ROWS_PER_CORE = D // NCORES          # 64
COLS_PER_CORE = S // NCORES          # 256
KT = D // 128                         # 4 contraction blocks
NS = S // 512                         # 4 chain-2 output column chunks
INV_V = 1.0 / V
BF = ml_dtypes.bfloat16

F32 = mybir.dt.float32
BF16 = mybir.dt.bfloat16
AF = mybir.ActivationFunctionType
ALU = mybir.AluOpType

# ---- tunables -------------------------------------------------------------
A_CHUNKS = [64, 64, 64, 48, 16]                # 128-col chunks per DMA tile


def _pad512(elems):
    """Round a bf16 element count up so the row pitch is a 512-byte multiple."""
    return ((elems * 2 + 511) // 512) * 512 // 2
A_BUFS = 4
A_WARMUP = 16
B_WARMUP = 16                                  # PE warm-up matmuls (HAM ramp)
B_WARM_FREE = 128

_CACHE = {}


def build_kernel_a():
    """Reduce kernel: gram-matrix trick on the tensor engine.

    Host sends the vuln shard TRANSPOSED and ones-augmented: 256 chunks of
    [128 j, 65] where cols 0:64 = v[j, row] and col 64 = 1. One long PSUM
    accumulation of chunk^T @ chunk[:, 0:64] yields [65, 64]: rows 0:64 are
    the gram matrix (diag = p), row 64 is msum. Host extracts diag/row.
    DMA-bound (~12 us); vector/scalar engines idle.
    """
    nc = bacc.Bacc("TRN2", target_bir_lowering=False, debug=False, num_devices=NCORES)

    NCH = V // 128                   # 256 chunks
    vins = [
        nc.dram_tensor(f"v{t}", [128, _pad512(ch * 65)], BF16, kind="ExternalInput")
        for t, ch in enumerate(A_CHUNKS)
    ]
    out_g = nc.dram_tensor("out_g", [65, 64], F32, kind="ExternalOutput")

    with tile.TileContext(nc) as tc:
        with (
            tc.tile_pool(name="vin", bufs=A_BUFS) as vin_pool,
            tc.tile_pool(name="small", bufs=1) as small_pool,
            tc.tile_pool(name="ps", bufs=1, space="PSUM") as ps_pool,
            tc.tile_pool(name="psw", bufs=1, space="PSUM") as psw_pool,
        ):
            # PE warm-up during preamble/first-tile DMA
            warm_b = small_pool.tile([128, 64], BF16, name="warm_b")
            nc.gpsimd.memset(warm_b[:], 0.001)
            ps_w = psw_pool.tile([64, 64], F32, name="ps_w")
            for i in range(A_WARMUP):
                nc.tensor.matmul(
                    ps_w[:], warm_b[:], warm_b[:],
                    start=(i == 0), stop=(i == A_WARMUP - 1),
                )

            vts = []
            maxw = _pad512(max(A_CHUNKS) * 65)
            for t, ch in enumerate(A_CHUNKS):
                w = _pad512(ch * 65)
                vt = vin_pool.tile([128, maxw], BF16, name=f"vt{t}", tag="vt")
                nc.sync.dma_start(vt[:, 0:w], vins[t][:])
                vts.append(vt)

            ps_g = ps_pool.tile([65, 64], F32, name="ps_g")
            done = 0
            for t, ch in enumerate(A_CHUNKS):
                for c in range(ch):
                    nc.tensor.matmul(
                        ps_g[:],
                        vts[t][:, 65 * c : 65 * c + 65],
                        vts[t][:, 65 * c : 65 * c + 64],
                        start=(done == 0),
                        stop=(done == NCH - 1),
                    )
                    done += 1

            gsb = small_pool.tile([65, 64], F32, name="gsb")
            nc.vector.tensor_copy(gsb[:], ps_g[:])
            nc.sync.dma_start(out_g[:], gsb[:])

    nc.compile()
    return nc


def build_kernel_b():
    """Main kernel: build R, two matmul chains, write transposed bf16 slabs."""
    nc = bacc.Bacc("TRN2", target_bir_lowering=False, debug=False, num_devices=NCORES)

    # pm_pp: per-partition layout, col kt = p[q+128kt], 4+kt = msum, 8+kt = msum^2
    pm_pp = nc.dram_tensor("pm_pp", [128, 3 * KT], F32, kind="ExternalInput")
    # combo (bf16, wide rows): [ msum^2 broadcast (512) | per-jt [E*bV | E*aV] ]
    meb_in = nc.dram_tensor("meb", [128, D + KT * 2 * COLS_PER_CORE], BF16,
                            kind="ExternalInput")
    efull = nc.dram_tensor("efull", [KT, 128, S], BF16, kind="ExternalInput")
    # transposed output slabs (host transposes back): out[:, cols] = slab.T
    out_re = nc.dram_tensor("out_re", [COLS_PER_CORE, S], BF16, kind="ExternalOutput")
    out_im = nc.dram_tensor("out_im", [COLS_PER_CORE, S], BF16, kind="ExternalOutput")

    rc = RECIP_APPROX_FAST_CONSTS
    CP = COLS_PER_CORE

    with tile.TileContext(nc) as tc:
        with (
            tc.tile_pool(name="epool", bufs=1) as e_pool,
            tc.tile_pool(name="small", bufs=1) as small_pool,
            tc.tile_pool(name="hb", bufs=2) as h_pool,
            tc.tile_pool(name="ost", bufs=4) as o_pool,
            tc.tile_pool(name="psA", bufs=1, space="PSUM") as psA,
            tc.tile_pool(name="psB", bufs=4, space="PSUM") as psB,
        ):
            # -------- input DMAs --------------------------------------------
            # sync: small early tensors; gpsimd: e0/e1; scalar (after h's): e2/e3
            pp = small_pool.tile([128, 3 * KT], F32, name="pp")
            nc.sync.dma_start(pp[:], pm_pp[:])
            meb_t = small_pool.tile([128, D + KT * 2 * CP], BF16, name="meb_t")
            nc.sync.dma_start(meb_t[:], meb_in[:])
            ms2_bc = meb_t[:, 0:D]
            ebea = [
                meb_t[:, D + 2 * CP * jt : D + 2 * CP * (jt + 1)]
                for jt in range(KT)
            ]

            e_sb = [
                e_pool.tile([128, S], BF16, name=f"e{kt}", tag=f"e{kt}")
                for kt in range(KT)
            ]

            # -------- PE warm-up (HAM ramp) during DMA/derivation -----------
            warm_b = small_pool.tile([128, B_WARM_FREE], BF16, name="warm_b")
            nc.gpsimd.memset(warm_b[:], 0.001)
            for kt in [3, 2, 1, 0]:
                nc.sync.dma_start(e_sb[kt][:], efull[kt])
            ps_w = psB.tile([128, 512], F32, name="ps_w", tag="o")
            for i in range(B_WARMUP):
                nc.tensor.matmul(
                    ps_w[:, 0:B_WARM_FREE], warm_b[:], warm_b[:],
                    start=(i == 0), stop=(i == B_WARMUP - 1),
                )

            # -------- tiny derived vectors ----------------------------------
            # b4n = sqrt(p); a4c = msum*sqrt(p)/V  (t_sb diag scalings)
            b4n = small_pool.tile([128, KT], F32, name="b4n")
            nc.scalar.activation(b4n[:], pp[:, 0:KT], AF.Sqrt)
            a4c = small_pool.tile([128, KT], F32, name="a4c")
            nc.vector.scalar_tensor_tensor(
                a4c[:], pp[:, KT : 2 * KT], INV_V, b4n[:], op0=ALU.mult, op1=ALU.mult
            )

            # -------- R build (descending jt) -------------------------------
            # per jt: scalar h = sqrt(ms2_i + ms2_j), vector recip -> bf16,
            # gpsimd masks the diagonal block.
            rt, rd = [None] * KT, [None] * KT
            for jt in [3, 2, 1, 0]:
                h = h_pool.tile([128, D], F32, name="h", tag="h")
                nc.scalar.activation(
                    h[:], ms2_bc, AF.Sqrt,
                    bias=pp[:, 2 * KT + jt : 2 * KT + jt + 1], scale=1.0,
                )
                rtj = e_pool.tile([128, D], BF16, name=f"rt{jt}", tag=f"rt{jt}")
                nc.vector._custom_dve(
                    RECIPROCAL_APPROX_FAST, out=rtj[:], in0=h[:],
                    s0=rc["s0"], s1=rc["s1"], imm2=rc["imm2"],
                )
                rt[jt] = rtj
                rdj = e_pool.tile([128, 128], BF16, name=f"rd{jt}", tag=f"rd{jt}")
                nc.gpsimd.affine_select(
                    out=rdj[:], in_=rtj[:, 128 * jt : 128 * (jt + 1)],
                    pattern=[[-1, 128]], compare_op=ALU.is_gt,
                    fill=0.0, base=0, channel_multiplier=1,
                )
                rd[jt] = rdj

            # -------- chain 1: T-blocks = R @ [E·bV | E·aV] -----------------
            ps_ts = [
                psA.tile([128, 2 * CP], F32, name=f"ps_t{it}", tag=f"t{it}", bufs=1)
                for it in range(KT)
            ]
            # descending jt: every group starts at jt==3; the diag (it==jt)
            # is each group's LAST contribution (stop), emitted after the
            # off-diagonal blocks so it can wait on rd[jt] without stalling.
            t_sb = [None] * KT
            for jt in [3, 2, 1, 0]:
                for it in range(jt):
                    nc.tensor.matmul(
                        ps_ts[it][:], rt[jt][:, 128 * it : 128 * (it + 1)], ebea[jt],
                        start=(jt == 3), stop=False,
                    )
                nc.tensor.matmul(
                    ps_ts[jt][:], rd[jt][:], ebea[jt],
                    start=(jt == 3), stop=True,
                )
                # T-block jt is complete: copy to SBUF with diag scalings
                tsb = e_pool.tile([128, 2 * CP], BF16, name=f"tsb{jt}", tag=f"tsb{jt}")
                nc.scalar.activation(
                    tsb[:, 0:CP], ps_ts[jt][:, 0:CP], AF.Copy,
                    scale=a4c[:, jt : jt + 1],
                )
                nc.vector.tensor_scalar_mul(
                    tsb[:, CP : 2 * CP], ps_ts[jt][:, CP : 2 * CP],
                    b4n[:, jt : jt + 1],
                )
                t_sb[jt] = tsb

            # -------- chain 2: out^T slabs = T^T @ E ------------------------
            # groups: (re, mc0), (re, mc1), (im, mc0), (im, mc1)
            groups = [
                (0, 0, out_re), (0, 1, out_re), (1, 0, out_im), (1, 1, out_im),
            ]
            cnt = 0
            for gi, (part, mc, out_t) in enumerate(groups):
                c0 = part * CP + mc * 128
                if gi % 2 == 0:
                    pso = [
                        psB.tile([128, 512], F32, name=f"pso{part}{mc}{sn}", tag="o")
                        for sn in range(NS)
                    ]
                else:
                    pso = [
                        psA.tile([128, 2 * CP], F32, name=f"pso{part}{mc}{sn}",
                                 tag=f"t{sn}", bufs=1)
                        for sn in range(NS)
                    ]
                for idx_it, it in enumerate([3, 2, 1, 0]):
                    for sn in range(NS):
                        nc.tensor.matmul(
                            pso[sn][:],
                            t_sb[it][:, c0 : c0 + 128],
                            e_sb[it][:, 512 * sn : 512 * (sn + 1)],
                            start=(idx_it == 0), stop=(idx_it == KT - 1),
                        )
                osb = o_pool.tile([128, S], BF16, name="osb", tag="osb")
                for sn in range(NS):
                    if cnt % 2 == 0:
                        nc.scalar.copy(osb[:, 512 * sn : 512 * (sn + 1)], pso[sn][:])
                    else:
                        nc.vector.tensor_copy(
                            osb[:, 512 * sn : 512 * (sn + 1)], pso[sn][:]
                        )
                    cnt += 1
                nc.sync.dma_start(out_t[mc * 128 : (mc + 1) * 128, :], osb[:])

    nc.compile()
    return nc


def _prepare_a_in_maps(vulns):
    vulns = np.asarray(vulns)
    NCH = V // 128
    in_maps = []
    for c in range(NCORES):
        vsh = vulns[c * ROWS_PER_CORE : (c + 1) * ROWS_PER_CORE]
        aug = np.empty((V, 65), dtype=BF)
        aug[:, 0:64] = vsh.T.astype(BF)
        aug[:, 64] = np.asarray(1.0, dtype=BF)
        # [NCH, 128, 65] -> [128, NCH*65] with chunk k at free cols [65k, 65k+65)
        v128 = aug.reshape(NCH, 128, 65).transpose(1, 0, 2).reshape(128, NCH * 65)
        m = {}
        off = 0
        for t, ch in enumerate(A_CHUNKS):
            w = ch * 65
            wp = _pad512(w)
            arr = np.zeros((128, wp), dtype=BF)
            arr[:, 0:w] = v128[:, off : off + w]
            m[f"v{t}"] = arr
            off += w
        in_maps.append(m)
    return in_maps


def _prepare_b_in_maps(embed_table, domain_ids, p_full, msum_full):
    embed_table = np.asarray(embed_table, dtype=np.float32)
    domain_ids = np.asarray(domain_ids).astype(np.int64)
    E = np.ascontiguousarray(embed_table[domain_ids])          # [512, 2048] f32
    e4 = np.ascontiguousarray(E.astype(BF).reshape(KT, 128, S))
    p64 = p_full.astype(np.float64)
    ms64 = msum_full.astype(np.float64)
    ms2 = (ms64 ** 2).astype(np.float32)
    bV = (float(V) * np.sqrt(p64)).astype(np.float32)          # V*sqrt(p)
    aV = (ms64 * np.sqrt(p64)).astype(np.float32)              # msum*sqrt(p)
    # per-partition layout [128, 12]
    pm_pp = np.empty((128, 3 * KT), dtype=np.float32)
    pm_pp[:, 0:KT] = p_full.reshape(KT, 128).T
    pm_pp[:, KT : 2 * KT] = msum_full.reshape(KT, 128).T
    pm_pp[:, 2 * KT : 3 * KT] = ms2.reshape(KT, 128).T
    CP = COLS_PER_CORE
    ms2_b = np.ascontiguousarray(np.broadcast_to(ms2.astype(BF), (128, D)))
    in_maps = []
    for c in range(NCORES):
        cols = slice(c * CP, (c + 1) * CP)
        Ec = E[:, cols]                                        # [512, 256] f32
        Eb = (Ec * bV[:, None]).astype(BF).reshape(KT, 128, CP)
        Ea = (Ec * aV[:, None]).astype(BF).reshape(KT, 128, CP)
        meb = np.empty((128, D + KT * 2 * CP), dtype=BF)
        meb[:, 0:D] = ms2_b
        for jt in range(KT):
            meb[:, D + 2 * CP * jt : D + 2 * CP * jt + CP] = Eb[jt]
            meb[:, D + 2 * CP * jt + CP : D + 2 * CP * (jt + 1)] = Ea[jt]
        in_maps.append(
            {"pm_pp": pm_pp, "meb": np.ascontiguousarray(meb), "efull": e4}
        )
    return in_maps


def kernel(vulns, embed_table, domain_ids, _trace=False):
    if "nc_a" not in _CACHE:
        _CACHE["nc_a"] = build_kernel_a()
    if "nc_b" not in _CACHE:
        _CACHE["nc_b"] = build_kernel_b()

    res_a = run_bass_kernel_spmd(
        _CACHE["nc_a"], _prepare_a_in_maps(vulns),
        core_ids=list(range(NCORES)), trace=_trace,
    )
    _CACHE["res_a"] = res_a
    idx = np.arange(ROWS_PER_CORE)
    p_full = np.concatenate(
        [res_a.results[c]["out_g"][idx, idx] for c in range(NCORES)]
    )
    msum_full = np.concatenate(
        [res_a.results[c]["out_g"][64, :] for c in range(NCORES)]
    )

    res_b = run_bass_kernel_spmd(
        _CACHE["nc_b"], _prepare_b_in_maps(embed_table, domain_ids, p_full, msum_full),
        core_ids=list(range(NCORES)), trace=_trace,
    )
    _CACHE["res_b"] = res_b

    out = np.empty((S, S), dtype=np.complex64)
    for c in range(NCORES):
        r = res_b.results[c]
        sl = slice(c * COLS_PER_CORE, (c + 1) * COLS_PER_CORE)
        out[:, sl] = (
            r["out_re"].astype(np.float32).T
            + 1j * r["out_im"].astype(np.float32).T
        )
    return out


if __name__ == "__main__":
    rng = np.random.default_rng(0)
    v = rng.standard_normal((D, V), dtype=np.float32)
    et = rng.standard_normal((D, S), dtype=np.float32)
    ids = np.arange(D, dtype=np.int32)
    out = kernel(v, et, ids)
    print(out.shape, out.dtype)
